# revision 16
# baseline (speedup 1.0000x reference)
"""Trainium2 Bass kernel for nn_DetectionLoss (nms_detection).

v2 architecture (data-parallel over batch, 8 cores x 4 images):
  - Pairwise grid C[q, j] computed per 128-query tile as f16 surfaces:
      * u_x, u_y (L1 halves) via ONE fused custom-DVE op each
        (U_ABS2: |t1 - q1| + |t2 - q2|), reading raw target broadcast
        tiles + per-partition query coords.
      * rank-1 terms (wq+Wb etc.) on ScalarE (bias-activation),
        intersection/enclose chain on DVE f16 tensor_tensor (2x mode),
        reciprocals via ScalarE exp(-ln(x)).
      * the final linear combination C = -cos + 5*L1 - 2*iou - 2*rat + 2
        is ACCUMULATED ON THE TENSOR ENGINE into PSUM: the class-cost
        matmul (pre-normalized region rows x negated-normalized text,
        repeated per target) plus lambda-scaled identity matmuls of
        u_x, u_y (x5) and iou, rat (x-2); the +2 rides the ScalarE
        PSUM->f16 evacuation bias.
  - Region rows are normalized on-device (Square+accum norms) and
    transposed to matmul lhsT layout with the DMA transpose engine.
  - C_out is written f16 and upconverted on host; scalar losses are
    per-core partial sums combined on host (host does no arithmetic on
    the cost volume beyond dtype conversion).
"""

import math
import os as _os
from contextlib import ExitStack

import numpy as np

import concourse.bass as bass
import concourse.bacc as bacc
import concourse.tile as tile
from concourse import mybir

# All activation funcs used (Abs/Exp/Ln/Relu/Square/Identity) live in one
# table set; restricting the chooser avoids per-op table thrash.
_orig_gat = bacc.get_activation_tables


def _gat_single_set(arch):
    t = _orig_gat(arch)
    name = "natural_log_exp_and_others"
    if name not in t:
        return t
    return {k: (v if k == name else set()) for k, v in t.items()}


bacc.get_activation_tables = _gat_single_set
from concourse.bass_utils import run_bass_kernel_spmd
from concourse.masks import make_identity

# ---- custom fused DVE op: out = |in0 - s0| + |in1 - s1| ----
import concourse.dve_ops as dve_ops
from concourse.dve_spec import Spec, Src0, Src1, C0, C1, Zero, maxx, lower
from concourse.dve_uop import DveOpSpec


def _register_dve_op(name, spec):
    if name in dve_ops._SUB_OPCODE_FOR_NAME:
        return next(o for o in dve_ops.OPS if o.name == name)
    shas = {}
    for ver in ("v3", "v4"):
        s = DveOpSpec(name=name, opcode=0, uops=lower(spec, ver=ver),
                      rd1_en=True)
        shas[ver] = s.sha(ver)
    op = dve_ops.DveOp(name, spec, subdim=False, uops_sha=shas)
    dve_ops.OPS.append(op)
    dve_ops.CUSTOM_DVE_SPECS[name] = spec
    dve_ops._SUB_OPCODE_FOR_NAME[name] = (
        max(dve_ops._SUB_OPCODE_FOR_NAME.values()) + 1)
    return op


_d1 = Src0 - C0
_d2 = Src1 - C1
U_ABS2 = _register_dve_op(
    "U_ABS2",
    Spec(body=maxx(_d1, Zero - _d1) + maxx(_d2, Zero - _d2),
         reference=lambda in0, in1, s0, s1, imm2:
         (np.abs(in0 - s0) + np.abs(in1 - s1)).astype(np.float32)))

from concourse.dve_spec import relu as _relu
INTER_RELU2 = _register_dve_op(
    "INTER_RELU2",
    Spec(body=_relu(Src0) * _relu(Src1),
         reference=lambda in0, in1, s0, s1, imm2:
         (np.maximum(in0, 0) * np.maximum(in1, 0)).astype(np.float32)))

# ---- problem constants (hardcoded; kernel.py must be self-contained) ----
B, Q, T, NNEG = 32, 900, 32, 10
RD, TD, PD = 256, 512, 256
TEMP = 0.07
NCORES = 8
BL = B // NCORES          # images per core = 4
QL = BL * Q               # queries per core = 3600
NT = (QL + 127) // 128    # 29 q-tiles per core
QP = NT * 128             # padded queries = 3712
J = B * T                 # 1024 targets (global)
ML = BL * T               # matched rows per core = 128
NL = BL * NNEG            # neg rows per core = 40
KT = TD // 128            # 4 k-chunks for text projection

F32 = mybir.dt.float32
F16 = mybir.dt.float16
I32 = mybir.dt.int32
AF = mybir.ActivationFunctionType
OP = mybir.AluOpType


def build_program(loop_tiles=NT, losses=True):
    nc = bacc.Bacc("TRN2", target_bir_lowering=False, debug=False,
                   num_devices=NCORES)

    def din(name, shape, dt=F32):
        return nc.dram_tensor(name, shape, dt, kind="ExternalInput").ap()

    def dout(name, shape, dt=F32):
        return nc.dram_tensor(name, shape, dt, kind="ExternalOutput").ap()

    ins = dict(
        reg_n=din("reg_n", [QP, RD]),          # local region feats (padded)
        bboxm=din("bboxm", [128, NT * 4]),     # local bbox, tile-marshalled
        bbox_rows=din("bbox_rows", [QP, 4]),   # local bbox, row layout
        clsm=din("clsm", [128, NT]),           # local cls logits, marshalled
        tgt_T=din("tgt_T", [4, J]),            # all target boxes, coord-major
        tgt_loc=din("tgt_loc", [ML, 4]),       # local target boxes row-major
        text_T=din("text_T", [TD, B]),         # all text emb, transposed
        text_rep_T=din("text_rep_T", [TD, ML]),  # local text, repeated+T
        Wt=din("Wt", [TD, PD]),
        bt_row=din("bt_row", [1, PD]),
        gidx=din("gidx", [ML, 1], I32),        # local matched row indices
        ngidx=din("ngidx", [NL, 1], I32),      # local negative row indices
    )
    outs = dict(
        C_out=dout("C_out", [QP, J], F16),
        loss_out=dout("loss_out", [64]),
        mask_scratch=dout("mask_scratch", [QP, 1]),
    )

    with tile.TileContext(nc) as tc:
        with ExitStack() as ctx:
            detection_kernel(ctx, tc, outs, ins, loop_tiles=loop_tiles,
                             losses=losses)
    nc.compile()
    return nc


def detection_kernel(ctx: ExitStack, tc: tile.TileContext, outs, ins,
                     loop_tiles=NT, losses=True):
    nc = tc.nc
    singles = ctx.enter_context(tc.tile_pool(name="singles", bufs=1))
    w1 = ctx.enter_context(tc.tile_pool(
        name="w1", bufs=int(_os.environ.get("K_W1BUFS", "2"))))
    w2 = ctx.enter_context(tc.tile_pool(
        name="w2", bufs=int(_os.environ.get("K_W2BUFS", "3"))))
    outp = ctx.enter_context(tc.tile_pool(
        name="outp", bufs=int(_os.environ.get("K_OUTBUFS", "2"))))
    psum = ctx.enter_context(tc.tile_pool(
        name="psum", bufs=int(_os.environ.get("K_PSBUFS", "2")), space="PSUM"))
    psum1 = ctx.enter_context(tc.tile_pool(name="psum1", bufs=1, space="PSUM"))

    # ---------------- identities ----------------
    ident = singles.tile([128, 128], F32)
    make_identity(nc, ident[:])
    I5 = singles.tile([128, 128], F16)      # +5 * I  (L1 weight)
    nc.vector.tensor_scalar(out=I5, in0=ident, scalar1=5.0, scalar2=None,
                            op0=OP.mult)
    In2 = singles.tile([128, 128], F16)     # -2 * I  (giou terms weight)
    nc.vector.tensor_scalar(out=In2, in0=ident, scalar1=-2.0, scalar2=None,
                            op0=OP.mult)

    junkR = singles.tile([128, RD], F32, tag="junkR")

    # ---------------- text: all-image normalized projection ----------------
    wt_s = singles.tile([128, KT * PD], F32)   # Wt k-chunks side by side
    for k in range(KT):
        nc.sync.dma_start(out=wt_s[:, k * PD:(k + 1) * PD],
                          in_=ins["Wt"][k * 128:(k + 1) * 128, :])
    bt_s = singles.tile([1, PD], F32)
    nc.sync.dma_start(out=bt_s, in_=ins["bt_row"][:, :])
    ones1 = singles.tile([1, B], F32)
    nc.vector.memset(ones1, 1.0)

    txtT_s = singles.tile([128, KT * B], F32)  # text_T k-chunks
    for k in range(KT):
        nc.sync.dma_start(out=txtT_s[:, k * B:(k + 1) * B],
                          in_=ins["text_T"][k * 128:(k + 1) * 128, :])
    ps_txt = psum1.tile([B, PD], F32, tag="ps_one")
    for k in range(KT):
        nc.tensor.matmul(out=ps_txt[:], lhsT=txtT_s[:, k * B:(k + 1) * B],
                         rhs=wt_s[:, k * PD:(k + 1) * PD],
                         start=(k == 0), stop=False)
    nc.tensor.matmul(out=ps_txt[:], lhsT=ones1[:], rhs=bt_s[:],
                     start=False, stop=True)
    txtp = singles.tile([B, PD], F32)
    nc.vector.tensor_copy(out=txtp, in_=ps_txt)
    junkB = junkR[0:B, :]
    n2t = singles.tile([B, 1], F32)
    nc.scalar.activation(out=junkB, in_=txtp, func=AF.Square,
                         accum_out=n2t[:])
    lnt = singles.tile([B, 1], F32)
    nc.scalar.activation(out=lnt, in_=n2t, func=AF.Ln)
    nit = singles.tile([B, 1], F32)
    nc.scalar.activation(out=nit, in_=lnt, func=AF.Exp, scale=-0.5)
    txtn = singles.tile([B, PD], F32)          # normalized text (positive)
    nc.vector.tensor_scalar(out=txtn, in0=txtp, scalar1=nit[:],
                            scalar2=None, op0=OP.mult)
    # transpose to [PD, B] = two [128, B] chunks (f32, also used by losses)
    txtT0 = singles.tile([128, B], F32)
    txtT1 = singles.tile([128, B], F32)
    for k, dst in ((0, txtT0), (1, txtT1)):
        ps_tt = psum1.tile([128, B], F32, tag="ps_one")
        nc.tensor.transpose(out=ps_tt[:], in_=txtn[:, k * 128:(k + 1) * 128],
                            identity=ident[0:B, 0:B])
        nc.vector.tensor_copy(out=dst, in_=ps_tt)
    # negated, repeated per-target rhs tiles for the class matmul (f16)
    rhsrep0 = singles.tile([128, J], F16)
    rhsrep1 = singles.tile([128, J], F16)
    for src, dst in ((txtT0, rhsrep0), (txtT1, rhsrep1)):
        src_b = bass.AP(tensor=src.tensor, offset=src.offset,
                        ap=[src.ap[0], [1, B], [0, T]])
        nc.vector.tensor_scalar(
            out=dst[:].rearrange("p (b t) -> p b t", t=T),
            in0=src_b, scalar1=-1.0, scalar2=None, op0=OP.mult)


    # ---------------- region rows resident + norms ----------------
    regrows = singles.tile([128, NT * RD], F32)
    n2c = singles.tile([128, NT], F32)
    for t in range(loop_tiles):
        sl = slice(t * RD, (t + 1) * RD)
        nc.sync.dma_start(out=regrows[:, sl],
                          in_=ins["reg_n"][t * 128:(t + 1) * 128, :])
        nc.scalar.activation(out=junkR, in_=regrows[:, sl], func=AF.Square,
                             accum_out=n2c[:, t:t + 1])

    # ---------------- target broadcast tiles (raw coords, f16) -----------
    def bcast_row(r):
        t = ins["tgt_T"]
        return bass.AP(tensor=t.tensor, offset=r * J, ap=[[0, 128], [1, J]])

    X1b = singles.tile([128, J], F16)
    Y1b = singles.tile([128, J], F16)
    X2b = singles.tile([128, J], F16)
    Y2b = singles.tile([128, J], F16)
    stgA = singles.tile([128, J], F32, tag="stgA")
    stgB = singles.tile([128, J], F32, tag="stgB")
    for i, (cb, r) in enumerate(((X1b, 0), (Y1b, 1), (X2b, 2), (Y2b, 3))):
        stg = stgA if i % 2 == 0 else stgB
        nc.sync.dma_start(out=stg, in_=bcast_row(r))
        nc.vector.tensor_copy(out=cb, in_=stg)
    Wb = singles.tile([128, J], F16)
    Hb = singles.tile([128, J], F16)
    AT4b = singles.tile([128, J], F16)
    nc.vector.tensor_sub(Wb, X2b, X1b)
    nc.vector.tensor_sub(Hb, Y2b, Y1b)
    nc.vector.scalar_tensor_tensor(out=AT4b, in0=Wb, scalar=4.0, in1=Hb,
                                   op0=OP.mult, op1=OP.mult)

    # ---------------- query-side per-partition scalars ----------------
    bbm = singles.tile([128, NT * 4], F32)
    nc.sync.dma_start(out=bbm, in_=ins["bboxm"][:, :])
    bbr = bbm[:].rearrange("p (t c) -> p t c", c=4)
    wqa = singles.tile([128, NT], F32)
    hqa = singles.tile([128, NT], F32)
    aq4a = singles.tile([128, NT], F32)
    nc.vector.tensor_sub(wqa, bbr[:, :, 2], bbr[:, :, 0])
    nc.vector.tensor_sub(hqa, bbr[:, :, 3], bbr[:, :, 1])
    nc.vector.scalar_tensor_tensor(out=aq4a, in0=wqa, scalar=4.0, in1=hqa,
                                   op0=OP.mult, op1=OP.mult)

    lnn2 = singles.tile([128, NT], F32)
    ninv = singles.tile([128, NT], F32)     # 1/||reg||
    GN = int(_os.environ.get("K_GN", str(loop_tiles)))
    for g0 in range(0, loop_tiles, GN):
        g1 = min(g0 + GN, loop_tiles)
        nc.scalar.activation(out=lnn2[:, g0:g1], in_=n2c[:, g0:g1],
                             func=AF.Ln)
        nc.scalar.activation(out=ninv[:, g0:g1], in_=lnn2[:, g0:g1],
                             func=AF.Exp, scale=-0.5)

    bias2 = singles.tile([128, 1], F32)
    nc.vector.memset(bias2, 2.0)

    # ================= main pairwise grid loop =================
    for t in range(loop_tiles):
        sl = slice(t * 128, (t + 1) * 128)
        x1q = bbr[:, t, 0:1]
        y1q = bbr[:, t, 1:2]
        x2q = bbr[:, t, 2:3]
        y2q = bbr[:, t, 3:4]
        wq = wqa[:, t:t + 1]
        hq = hqa[:, t:t + 1]
        aq4 = aq4a[:, t:t + 1]

        # normalized region lhsT chunks via DMA transpose
        regn16 = w2.tile([128, RD], F16, tag="regn16")
        nc.vector.tensor_scalar(out=regn16,
                                in0=regrows[:, t * RD:(t + 1) * RD],
                                scalar1=ninv[:, t:t + 1], scalar2=None,
                                op0=OP.mult)
        rgs0 = w2.tile([128, 128], F16, tag="rgs0")
        rgs1 = w2.tile([128, 128], F16, tag="rgs1")
        nc.sync.dma_start_transpose(out=rgs0[:], in_=regn16[:, 0:128])
        nc.sync.dma_start_transpose(out=rgs1[:], in_=regn16[:, 128:256])

        # PSUM accumulator: C = -cos + 5*(u_x+u_y) - 2*(iou+rat)  [+2 at evac]
        ps = psum.tile([128, J], F32, tag="ps_C")
        H = J // 2
        def mm(lhsT, rhs, start=False, stop=False):
            for h in range(2):
                nc.tensor.matmul(out=ps[:, h * H:(h + 1) * H], lhsT=lhsT,
                                 rhs=rhs[:, h * H:(h + 1) * H],
                                 start=start, stop=stop)
        mm(rgs0[:], rhsrep0[:], start=True)
        mm(rgs1[:], rhsrep1[:])

        # L1 halves: one fused DVE op each
        u_x = w1.tile([128, J], F16, tag="u_x")
        u_y = w1.tile([128, J], F16, tag="u_y")
        nc.vector._custom_dve(U_ABS2, out=u_x[:], in0=X1b[:], in1=X2b[:],
                              s0=x1q, s1=x2q)
        nc.vector._custom_dve(U_ABS2, out=u_y[:], in0=Y1b[:], in1=Y2b[:],
                              s0=y1q, s1=y2q)
        mm(I5[:], u_x[:])
        mm(I5[:], u_y[:])

        # rank-1 sums on ScalarE
        sW = w1.tile([128, J], F16, tag="sW")
        sH = w1.tile([128, J], F16, tag="sH")
        sA = w1.tile([128, J], F16, tag="sA")
        nc.scalar.activation(out=sW, in_=Wb, func=AF.Identity, bias=wq)
        nc.scalar.activation(out=sH, in_=Hb, func=AF.Identity, bias=hq)
        nc.scalar.activation(out=sA, in_=AT4b, func=AF.Identity, bias=aq4)

        # intersection / enclose chain (f16 TT, 2x mode)
        wi2 = w1.tile([128, J], F16, tag="wi2")
        hi2 = w1.tile([128, J], F16, tag="hi2")
        we2 = w1.tile([128, J], F16, tag="we2")
        he2 = w1.tile([128, J], F16, tag="he2")
        nc.vector.tensor_sub(wi2, sW, u_x)
        nc.vector.tensor_sub(hi2, sH, u_y)
        nc.vector.tensor_add(we2, sW, u_x)
        nc.vector.tensor_add(he2, sH, u_y)
        inter = w1.tile([128, J], F16, tag="inter")
        nc.vector._custom_dve(INTER_RELU2, out=inter[:], in0=wi2[:],
                              in1=hi2[:])
        ue = w1.tile([128, 2 * J], F16, tag="ue")   # [union | enclose]
        nc.vector.tensor_sub(ue[:, 0:J], sA, inter)
        nc.vector.tensor_mul(ue[:, J:2 * J], we2, he2)

        # reciprocals on ScalarE: 1/x = exp(-ln(x)), both halves in one op
        lnue = w1.tile([128, 2 * J], F16, tag="lnue")
        nc.scalar.activation(out=lnue, in_=ue, func=AF.Ln)
        rue = w1.tile([128, 2 * J], F16, tag="rue")
        nc.scalar.activation(out=rue, in_=lnue, func=AF.Exp, scale=-1.0)

        iou = w1.tile([128, J], F16, tag="iou")
        rat = w1.tile([128, J], F16, tag="rat")
        nc.vector.tensor_mul(iou, inter, rue[:, 0:J])
        nc.vector.tensor_mul(rat, ue[:, 0:J], rue[:, J:2 * J])
        mm(In2[:], iou[:])
        mm(In2[:], rat[:], stop=True)

        # evacuate: C = psum + 2, f16
        Cot = outp.tile([128, J], F16, tag="Cot")
        nc.scalar.activation(out=Cot, in_=ps, func=AF.Identity,
                             bias=bias2[:])
        nc.sync.dma_start(out=outs["C_out"][sl, :], in_=Cot[:])

    if losses:
        emit_losses(nc, tc, singles, psum1, outs, ins, ident, junkR,
                    wt_s, bt_s, txtT0, txtT1)


def emit_losses(nc, tc, singles, psum1, outs, ins, ident, junkR,
                wt_s, bt_s, txtT0, txtT1):
    if True:
        # ---------------- text: local repeated normalized projection --------
        txtRT_s = singles.tile([128, KT * ML], F32)
        for k in range(KT):
            nc.sync.dma_start(out=txtRT_s[:, k * ML:(k + 1) * ML],
                              in_=ins["text_rep_T"][k * 128:(k + 1) * 128, :])
        onesM = singles.tile([1, ML], F32)
        nc.vector.memset(onesM, 1.0)
        ps_txr = psum1.tile([ML, PD], F32, tag="ps_one")
        for k in range(KT):
            nc.tensor.matmul(out=ps_txr[:], lhsT=txtRT_s[:, k * ML:(k + 1) * ML],
                             rhs=wt_s[:, k * PD:(k + 1) * PD],
                             start=(k == 0), stop=False)
        nc.tensor.matmul(out=ps_txr[:], lhsT=onesM[:], rhs=bt_s[:],
                         start=False, stop=True)
        txrp = singles.tile([ML, PD], F32)
        nc.vector.tensor_copy(out=txrp, in_=ps_txr)
        junkM = junkR[:, :]
        n2r = singles.tile([ML, 1], F32)
        nc.scalar.activation(out=junkM, in_=txrp, func=AF.Square,
                             accum_out=n2r[:])
        lnr = singles.tile([ML, 1], F32)
        nc.scalar.activation(out=lnr, in_=n2r, func=AF.Ln)
        nir = singles.tile([ML, 1], F32)
        nc.scalar.activation(out=nir, in_=lnr, func=AF.Exp, scale=-0.5)
        txtrep = singles.tile([ML, PD], F32)   # normalized, pre-scaled 1/TEMP
        nc.vector.tensor_scalar(out=txtrep, in0=txrp, scalar1=nir[:],
                                scalar2=1.0 / TEMP, op0=OP.mult, op1=OP.mult)

        # ---------------- gathers: pos / neg regions, matched boxes ----------
        gidx_t = singles.tile([ML, 1], I32)
        nc.sync.dma_start(out=gidx_t, in_=ins["gidx"][:, :])
        ngidx_t = singles.tile([NL, 1], I32)
        nc.sync.dma_start(out=ngidx_t, in_=ins["ngidx"][:, :])

        pos = singles.tile([ML, RD], F32)
        nc.gpsimd.indirect_dma_start(
            out=pos[:], out_offset=None, in_=ins["reg_n"][:, :],
            in_offset=bass.IndirectOffsetOnAxis(ap=gidx_t[:, 0:1], axis=0))
        neg = singles.tile([NL, RD], F32)
        nc.gpsimd.indirect_dma_start(
            out=neg[:], out_offset=None, in_=ins["reg_n"][:, :],
            in_offset=bass.IndirectOffsetOnAxis(ap=ngidx_t[:, 0:1], axis=0))
        sbx = singles.tile([ML, 4], F32)
        nc.gpsimd.indirect_dma_start(
            out=sbx[:], out_offset=None, in_=ins["bbox_rows"][:, :],
            in_offset=bass.IndirectOffsetOnAxis(ap=gidx_t[:, 0:1], axis=0))

        # normalize pos / neg region rows
        n2p = singles.tile([ML, 1], F32)
        nc.scalar.activation(out=junkM, in_=pos, func=AF.Square,
                             accum_out=n2p[:])
        lnp = singles.tile([ML, 1], F32)
        nc.scalar.activation(out=lnp, in_=n2p, func=AF.Ln)
        nip = singles.tile([ML, 1], F32)
        nc.scalar.activation(out=nip, in_=lnp, func=AF.Exp, scale=-0.5)
        posn = singles.tile([ML, RD], F32)
        nc.vector.tensor_scalar(out=posn, in0=pos, scalar1=nip[:], scalar2=None,
                                op0=OP.mult)
        n2n = singles.tile([NL, 1], F32)
        junkN = junkR[0:NL, :]
        nc.scalar.activation(out=junkN, in_=neg, func=AF.Square,
                             accum_out=n2n[:])
        lnn = singles.tile([NL, 1], F32)
        nc.scalar.activation(out=lnn, in_=n2n, func=AF.Ln)
        nin = singles.tile([NL, 1], F32)
        nc.scalar.activation(out=nin, in_=lnn, func=AF.Exp, scale=-0.5)
        negn = singles.tile([NL, RD], F32)
        nc.vector.tensor_scalar(out=negn, in0=neg, scalar1=nin[:], scalar2=None,
                                op0=OP.mult)

        # partials tile: cols = [sp_sum, xtgt_sum, l1_sum, g2_sum, diag_sum, ..]
        P5 = singles.tile([128, 8], F32)
        nc.vector.memset(P5, 0.0)

        # diag: rowwise dot(txtrep, posn); 1/TEMP pre-folded into txtrep
        nc.vector.tensor_tensor(out=junkM, in0=txtrep, in1=posn, op=OP.mult)
        nc.vector.tensor_reduce(out=P5[:, 4:5], in_=junkM,
                                axis=mybir.AxisListType.X, op=OP.add)

        # ---------------- loss_sim column block: S = txtn @ [posn|negn]^T ----
        arT0 = singles.tile([128, ML + NL], F32)
        arT1 = singles.tile([128, ML + NL], F32)
        for k, dst in ((0, arT0), (1, arT1)):
            ps_a = psum1.tile([128, ML], F32, tag="ps_one")
            nc.tensor.transpose(out=ps_a[:], in_=posn[:, k * 128:(k + 1) * 128],
                                identity=ident[:])
            nc.vector.tensor_copy(out=dst[:, 0:ML], in_=ps_a)
            ps_b = psum1.tile([128, NL], F32, tag="ps_one")
            nc.tensor.transpose(out=ps_b[:], in_=negn[:, k * 128:(k + 1) * 128],
                                identity=ident[0:NL, 0:NL])
            nc.vector.tensor_copy(out=dst[:, ML:ML + NL], in_=ps_b)
        ps_s = psum1.tile([B, ML + NL], F32, tag="ps_one")
        nc.tensor.matmul(out=ps_s[:], lhsT=txtT0[:], rhs=arT0[:], start=True,
                         stop=False)
        nc.tensor.matmul(out=ps_s[:], lhsT=txtT1[:], rhs=arT1[:], start=False,
                         stop=True)
        expS = singles.tile([B, ML + NL], F32)
        expsum = singles.tile([B, 1], F32)
        nc.scalar.activation(out=expS, in_=ps_s, func=AF.Exp, scale=1.0 / TEMP,
                             accum_out=expsum[:])

        # ---------------- cls loss partials ----------------
        clst = singles.tile([128, NT], F32)
        nc.sync.dma_start(out=clst, in_=ins["clsm"][:, :])
        # softplus(x) = relu(x) + ln(1 + exp(-|x|)) -- stable, sim-supported
        spa = singles.tile([128, NT], F32)
        nc.scalar.activation(out=spa, in_=clst, func=AF.Abs)
        spe = singles.tile([128, NT], F32)
        nc.scalar.activation(out=spe, in_=spa, func=AF.Exp, scale=-1.0)
        nc.vector.tensor_scalar(out=spe, in0=spe, scalar1=1.0, scalar2=None,
                                op0=OP.add)
        spl = singles.tile([128, NT], F32)
        nc.scalar.activation(out=spl, in_=spe, func=AF.Ln)
        spr = singles.tile([128, NT], F32)
        nc.vector.tensor_scalar(out=spr, in0=clst, scalar1=0.0, scalar2=None,
                                op0=OP.max)
        junkT = singles.tile([128, NT], F32, tag="junkT")
        nc.vector.tensor_tensor(out=junkT, in0=spl, in1=spr, op=OP.add)
        nc.vector.tensor_reduce(out=P5[:, 0:1], in_=junkT,
                                axis=mybir.AxisListType.X, op=OP.add)
        # scatter ones -> mask at matched query rows (dup-safe), read back
        zeroT = singles.tile([128, NT], F32, tag="zeroT")
        nc.vector.memset(zeroT, 0.0)
        msk_dst = bass.AP(tensor=outs["mask_scratch"].tensor, offset=0,
                          ap=[[1, 128], [128, NT]])
        nc.sync.dma_start(out=msk_dst, in_=zeroT[:])
        onesML = singles.tile([ML, 1], F32)
        nc.vector.memset(onesML, 1.0)
        nc.gpsimd.indirect_dma_start(
            out=outs["mask_scratch"][:, :],
            out_offset=bass.IndirectOffsetOnAxis(ap=gidx_t[:, 0:1], axis=0),
            in_=onesML[:], in_offset=None)
        maskt = singles.tile([128, NT], F32)
        msk_src = bass.AP(tensor=outs["mask_scratch"].tensor, offset=0,
                          ap=[[1, 128], [128, NT]])
        nc.sync.dma_start(out=maskt, in_=msk_src)
        junkT2 = singles.tile([128, NT], F32, tag="junkT2")
        nc.vector.tensor_tensor(out=junkT2, in0=maskt, in1=clst, op=OP.mult)
        nc.vector.tensor_reduce(out=P5[:, 1:2], in_=junkT2,
                                axis=mybir.AxisListType.X, op=OP.add)

        # ---------------- matched-pair L1 and GIoU ----------------
        tl = singles.tile([ML, 4], F32)
        nc.sync.dma_start(out=tl, in_=ins["tgt_loc"][:, :])
        d4 = singles.tile([ML, 4], F32)
        nc.vector.tensor_sub(d4, sbx, tl)
        junk4 = singles.tile([ML, 4], F32, tag="junk4")
        nc.scalar.activation(out=junk4, in_=d4, func=AF.Abs,
                             accum_out=P5[:, 2:3])

        lt2 = singles.tile([ML, 2], F32)
        rb2 = singles.tile([ML, 2], F32)
        nc.vector.tensor_tensor(out=lt2, in0=sbx[:, 0:2], in1=tl[:, 0:2],
                                op=OP.max)
        nc.vector.tensor_tensor(out=rb2, in0=sbx[:, 2:4], in1=tl[:, 2:4],
                                op=OP.min)
        wh2 = singles.tile([ML, 2], F32)
        nc.vector.tensor_sub(wh2, rb2, lt2)
        whr = singles.tile([ML, 2], F32)
        nc.vector.tensor_scalar(out=whr, in0=wh2, scalar1=0.0, scalar2=None,
                                op0=OP.max)
        inter1 = singles.tile([ML, 1], F32)
        nc.vector.tensor_mul(inter1, whr[:, 0:1], whr[:, 1:2])
        wa = singles.tile([ML, 1], F32)
        ha = singles.tile([ML, 1], F32)
        a1 = singles.tile([ML, 1], F32)
        nc.vector.tensor_sub(wa, sbx[:, 2:3], sbx[:, 0:1])
        nc.vector.tensor_sub(ha, sbx[:, 3:4], sbx[:, 1:2])
        nc.vector.tensor_mul(a1, wa, ha)
        wb_ = singles.tile([ML, 1], F32)
        hb_ = singles.tile([ML, 1], F32)
        a2 = singles.tile([ML, 1], F32)
        nc.vector.tensor_sub(wb_, tl[:, 2:3], tl[:, 0:1])
        nc.vector.tensor_sub(hb_, tl[:, 3:4], tl[:, 1:2])
        nc.vector.tensor_mul(a2, wb_, hb_)
        uni = singles.tile([ML, 1], F32)
        nc.vector.scalar_tensor_tensor(out=uni, in0=inter1, scalar=-1.0,
                                       in1=a1, op0=OP.mult, op1=OP.add)
        nc.vector.tensor_add(uni, uni, a2)
        lte = singles.tile([ML, 2], F32)
        rbe = singles.tile([ML, 2], F32)
        nc.vector.tensor_tensor(out=lte, in0=sbx[:, 0:2], in1=tl[:, 0:2],
                                op=OP.min)
        nc.vector.tensor_tensor(out=rbe, in0=sbx[:, 2:4], in1=tl[:, 2:4],
                                op=OP.max)
        whe = singles.tile([ML, 2], F32)
        nc.vector.tensor_sub(whe, rbe, lte)
        enc = singles.tile([ML, 1], F32)
        nc.vector.tensor_mul(enc, whe[:, 0:1], whe[:, 1:2])
        lnu2 = singles.tile([ML, 1], F32)
        nc.scalar.activation(out=lnu2, in_=uni, func=AF.Ln)
        ru2 = singles.tile([ML, 1], F32)
        nc.scalar.activation(out=ru2, in_=lnu2, func=AF.Exp, scale=-1.0)
        lne2 = singles.tile([ML, 1], F32)
        nc.scalar.activation(out=lne2, in_=enc, func=AF.Ln)
        re2 = singles.tile([ML, 1], F32)
        nc.scalar.activation(out=re2, in_=lne2, func=AF.Exp, scale=-1.0)
        t1g = singles.tile([ML, 1], F32)
        t2g = singles.tile([ML, 1], F32)
        nc.vector.tensor_mul(t1g, inter1, ru2)
        nc.vector.tensor_mul(t2g, uni, re2)
        junk1 = singles.tile([ML, 1], F32, tag="junk1")
        nc.vector.tensor_tensor(out=junk1, in0=t1g, in1=t2g, op=OP.add)
        nc.vector.tensor_reduce(out=P5[:, 3:4], in_=junk1,
                                axis=mybir.AxisListType.X, op=OP.add)

        # ---------------- reduce partials across partitions, write out ------
        ones128 = singles.tile([128, 1], F32)
        nc.vector.memset(ones128, 1.0)
        ps_l = psum1.tile([8, 1], F32, tag="ps_one")
        nc.tensor.matmul(out=ps_l[:], lhsT=P5[:], rhs=ones128[:], start=True,
                         stop=True)
        ls8 = singles.tile([8, 1], F32)
        nc.vector.tensor_copy(out=ls8, in_=ps_l)
        nc.sync.dma_start(out=outs["loss_out"][0:8], in_=ls8[:])
        nc.sync.dma_start(out=outs["loss_out"][8:8 + B], in_=expsum[:])


_NC_CACHE = None


def _get_program():
    global _NC_CACHE
    if _NC_CACHE is None:
        _NC_CACHE = build_program()
    return _NC_CACHE


def make_in_maps(inputs):
    """Shard + marshal FULL inputs into 8 per-core input maps."""
    rf = np.ascontiguousarray(inputs["region_features"], np.float32)
    bb = np.ascontiguousarray(inputs["bbox_pred"], np.float32)
    cp = np.ascontiguousarray(inputs["cls_pred"], np.float32)
    tb = np.ascontiguousarray(inputs["tgt_boxes"], np.float32)
    te = np.ascontiguousarray(inputs["text_embeddings"], np.float32)
    pi = np.ascontiguousarray(inputs["pred_idx"], np.int32)
    ni = np.ascontiguousarray(inputs["neg_idx"], np.int32)
    Wt = np.ascontiguousarray(inputs["Wt"], np.float32)
    bt = np.ascontiguousarray(inputs["bt"], np.float32)

    tgt_T = np.ascontiguousarray(tb.reshape(J, 4).T)          # [4, J]
    text_T = np.ascontiguousarray(te.T)                       # [TD, B]
    bt_row = bt.reshape(1, PD)

    in_maps = []
    for k in range(NCORES):
        gb = slice(k * BL, (k + 1) * BL)
        reg = rf[gb].reshape(QL, RD)
        reg_n = np.zeros((QP, RD), np.float32)
        reg_n[:QL] = reg
        bbox = bb[gb].reshape(QL, 4)
        bbox_rows = np.zeros((QP, 4), np.float32)
        bbox_rows[:QL] = bbox
        bboxm = np.ascontiguousarray(
            bbox_rows.reshape(NT, 128, 4).transpose(1, 0, 2).reshape(128, NT * 4))
        cls = np.full(QP, -50.0, np.float32)
        cls[:QL] = cp[gb].reshape(QL)
        clsm = np.ascontiguousarray(cls.reshape(NT, 128).T)
        tgt_loc = np.ascontiguousarray(tb[gb].reshape(ML, 4))
        text_rep = np.repeat(te[gb], T, axis=0)               # [ML, TD]
        text_rep_T = np.ascontiguousarray(text_rep.T)
        loc_off = (np.arange(BL, dtype=np.int32) * Q)[:, None]
        gidx = (pi[gb] + loc_off).reshape(ML, 1).astype(np.int32)
        ngidx = (ni[gb] + loc_off).reshape(NL, 1).astype(np.int32)
        in_maps.append(dict(
            reg_n=reg_n, bboxm=bboxm, bbox_rows=bbox_rows,
            clsm=clsm, tgt_T=tgt_T, tgt_loc=tgt_loc, text_T=text_T,
            text_rep_T=text_rep_T, Wt=Wt, bt_row=bt_row, gidx=gidx,
            ngidx=ngidx))
    return in_maps


def combine(results):
    """Combine per-core outputs into the full flat reference output."""
    C = np.empty((B, Q, J), np.float32)
    sp = xt = l1s = g2s = dg = 0.0
    expsum = np.zeros(B, np.float64)
    for k, r in enumerate(results):
        C[k * BL:(k + 1) * BL] = (
            r["C_out"][:QL].astype(np.float32).reshape(BL, Q, J))
        lo = r["loss_out"].astype(np.float64)
        sp += lo[0]
        xt += lo[1]
        l1s += lo[2]
        g2s += lo[3]
        dg += lo[4]
        expsum += lo[8:8 + B]
    loss_cls = 2.0 * (sp - xt) / (B * Q)
    loss_l1 = 5.0 * l1s / (B * T * 4)
    giou_mean = (g2s - B * T) / (B * T)
    loss_giou = 2.0 * (1.0 - giou_mean)
    loss_sim = np.mean(np.log(expsum)) - dg / (B * T)
    losses = np.array([loss_cls, loss_l1, loss_giou, loss_sim], np.float32)
    return np.concatenate([C.reshape(-1), losses])


def run(inputs, trace=False, **kw):
    nc = _get_program()
    in_maps = make_in_maps(inputs)
    try:
        res = run_bass_kernel_spmd(nc, in_maps, core_ids=list(range(NCORES)),
                                   trace=trace, **kw)
    except ModuleNotFoundError:
        res = run_bass_kernel_spmd(nc, in_maps, core_ids=list(range(NCORES)),
                                   trace=False, **kw)
    return combine(res.results), res


def kernel(**inputs) -> np.ndarray:
    out, _ = run(inputs)
    return out


if __name__ == "__main__":
    import reference
    inputs = {k: np.asarray(v) for k, v in reference.setup_inputs().items()}
    out = kernel(**inputs)
    exp = np.asarray(reference.reference(**inputs))
    err = np.abs(out - exp)
    scale = np.abs(exp).max()
    print("max abs err:", err.max(), " scale:", scale,
          " rel:", err.max() / scale)
